# revision 12
# baseline (speedup 1.0000x reference)
"""Combined CE + Dice + Focal-Tversky segmentation loss on 8 Trainium2 cores.

Layout: pure data parallel, 2 images per core. Per image, class planes are
packed in "class pair" tiles [128, 4096] bf16: pair j holds class 2j on
partitions 0-63 and class 2j+1 on partitions 64-127; partition p%64 holds
pixels [(p%64)*4096, (p%64+1)*4096).

Engine split (v2, tuned to measured TRN2 rates):
  ACT    Exp(logits), Ln(denominator)+CE-accum, Exp(-lse) -> 1/S
  PE     cross-class+cross-half softmax denominator sums only (PSUM accum)
  DVE    is_equal masks (tensor_scalar, 4x mode, fused t_sum accum),
         q = E*R2 and qm = q*M products (tensor_tensor, 2x mode),
         p_sum/TP column accumulators (tensor_scalar 4x accum passes)
  GPSIMD unused: its software-emulated ops run ~30us/instr and starve DVE
Host: CE's sum of target-class logits (gather on the bf16-rounded logits)
and the final scalar combine in float64.
"""

import os
import shutil
import sys
import tempfile

sys.path.insert(0, "/opt/trn_rl_repo")

import numpy as np

import concourse.bacc as bacc
import concourse.mybir as mybir
import concourse.tile as tile
from concourse.bass_utils import run_bass_kernel_spmd
from operator import add as _add

import concourse.dve_ops as _dom
from concourse.dve_ops import TENSOR_TENSOR_REDUCE as TTR_OP
from concourse.dve_ops import DveOp as _DveOp
from concourse.dve_spec import C0 as _C0, C1 as _C1, Spec as _Spec, \
    Src0 as _Src0, Src1 as _Src1, Zero as _Zero, select as _select


def _eq_ref(in0, in1, c0, c1, c2):
    body = np.where((in0.astype(np.float32) >= c0) & (in0 <= c1),
                    in1.astype(np.float32), 0.0)
    return body, body.sum(axis=-1, keepdims=True)


# Fused "select(lo<=tg<=hi, q, 0)" with free-dim sum accumulator: one DVE
# pass produces the masked-probability TP contribution without a separate
# mask tile.  Registered into the ant-dve custom-op table at import time.
EQSEL_OP = _DveOp(
    "EQ_SELECT_REDUCE_LOSS",
    _Spec(
        body=_select((_Src0 >= _C0) & (_Src0 <= _C1), _Src1, _Zero),
        accum=_add,
        accum_init=_Zero,
        reference=_eq_ref,
    ),
    subdim=False,
    uops_sha={"v3": "c6f19abea79fcfd4"},
)
if EQSEL_OP.name not in _dom._SUB_OPCODE_FOR_NAME:
    _dom.OPS.append(EQSEL_OP)
    _dom.CUSTOM_DVE_SPECS[EQSEL_OP.name] = EQSEL_OP.spec
    _dom._SUB_OPCODE_FOR_NAME[EQSEL_OP.name] = (
        _dom._CUSTOM_DVE_ROW_BASE + len(_dom.OPS) - 1)

B, C, H, W = 16, 6, 512, 512
NCORES = 8
BPC = B // NCORES  # images per core
HWPX = H * W  # 262144 pixels per image
PHALF = 64
FD = HWPX // PHALF  # 4096 free-dim columns per image
NPAIR = C // 2  # 3 class-pair tiles

CE_W, DICE_W, FT_W = 0.4, 0.4, 0.2
FT_ALPHA, FT_BETA, FT_GAMMA = 0.7, 0.3, 1.33

BF16 = mybir.dt.bfloat16
F32 = mybir.dt.float32
AF = mybir.ActivationFunctionType
ALU = mybir.AluOpType
NPBF16 = mybir.dt.np(BF16)

# tuning knobs
CH = 2048  # chunk free size
SUB = 512  # PSUM-bank sub-chunk for denominator matmuls
SCOL = 32  # stats columns reserved per image


def _pin_act_tables():
    """Point walrus at an act_info.json whose only exp/ln-bearing set is the
    combined natural_log_exp_and_others, so interleaved Exp/Ln ACTIVATEs do
    not thrash ACT_TABLE_LOADs."""
    if os.environ.get("BASS_ACT_ROOT_JSON_PATH"):
        return
    try:
        import json

        from neuronxcc.driver.Job import Job
        from neuronxcc.driver.jobs.support.FindActInfo import findActInfoFile

        src = findActInfoFile(Job.getPackageDir(), "gen3")
        if not src or not os.path.exists(src):
            return
        srcdir = os.path.dirname(src)
        dst = os.path.join(tempfile.gettempdir(), "act_root_lnexp")
        if not os.path.isdir(dst):
            tmp = dst + ".tmp"
            shutil.rmtree(tmp, ignore_errors=True)
            shutil.copytree(srcdir, tmp)
            info = json.load(open(os.path.join(tmp, "act_info.json")))
            keep = [s for s in info["act_func_sets"]
                    if s["name"] not in ("exp_and_others", "natural_log")]
            first = [s for s in keep if s["name"] == "natural_log_exp_and_others"]
            rest = [s for s in keep if s["name"] != "natural_log_exp_and_others"]
            info["act_func_sets"] = first + rest
            json.dump(info, open(os.path.join(tmp, "act_info.json"), "w"))
            os.replace(tmp, dst)
        os.environ["BASS_ACT_ROOT_JSON_PATH"] = os.path.join(dst, "act_info.json")
    except Exception:
        pass  # fall back to default tables; correctness unaffected


PIN_ACT_SET = os.environ.get("KPIN", "0") == "1"


def _build(fd=FD, ch=CH, sub=SUB, bpc=BPC):
    if PIN_ACT_SET:
        _pin_act_tables()
    nch = fd // ch
    nsub = ch // sub
    nc = bacc.Bacc("TRN2", target_bir_lowering=False, debug=False,
                   enable_asserts=False, num_devices=NCORES)

    lg_d = nc.dram_tensor("lg", [bpc, NPAIR, 128, fd], BF16, kind="ExternalInput")
    tg_d = nc.dram_tensor("tg", [bpc, 128, fd], BF16, kind="ExternalInput")
    wd_d = nc.dram_tensor("wd", [128, 128], BF16, kind="ExternalInput")
    cb_d = nc.dram_tensor("cb", [128, NPAIR, 2], F32, kind="ExternalInput")
    out_d = nc.dram_tensor("out", [128, SCOL * bpc], F32, kind="ExternalOutput")

    with tile.TileContext(nc) as tc:
        with (
            tc.tile_pool(name="inp", bufs=1) as inp,
            tc.tile_pool(name="wk", bufs=2) as wk,
            tc.tile_pool(name="acc", bufs=1) as accp,
            tc.tile_pool(name="ps", bufs=1, space="PSUM") as ps,
        ):
            wd_t = inp.tile([128, 128], BF16, tag="wd")
            nc.sync.dma_start(wd_t[:], wd_d.ap())
            cb_t = inp.tile([128, NPAIR, 2], F32, tag="cb")
            nc.sync.dma_start(cb_t[:], cb_d.ap())

            lg_t = inp.tile([128, bpc, NPAIR, fd], BF16, tag="lg")
            tg_t = inp.tile([128, bpc, fd], BF16, tag="tg")
            chunks = {b: ([(0, 512), (512, 3584)] if b == 0
                          else [(0, 4096)]) for b in range(bpc)}
            for b in range(bpc):
                for base, ch_i in chunks[b]:
                    for j in range(NPAIR):
                        nc.sync.dma_start(
                            lg_t[:, b, j, base:base + ch_i],
                            lg_d.ap()[b, j, :, base:base + ch_i])
                    if base == 0:
                        nc.sync.dma_start(tg_t[:, b, :], tg_d.ap()[b])

            stats = accp.tile([128, SCOL * bpc], F32, tag="stats")
            nc.vector.memset(stats[:], 0.0)
            dummy = accp.tile([128, fd], BF16, tag="dummy")
            s2all = ps.tile([128, fd], F32, tag="s2")

            for b in range(bpc):
                sb = SCOL * b
                subs_done = 0
                for chi, (base, ch_i) in enumerate(chunks[b]):
                    sl_ch = slice(base, base + ch_i)
                    nsub_i = ch_i // sub
                    # --- ACT: exponentials of all class pairs ---
                    E = []
                    for j in range(NPAIR):
                        Ej = wk.tile([128, fd], BF16, tag=f"E{j}")
                        nc.scalar.activation(Ej[:, :ch_i], lg_t[:, b, j, sl_ch],
                                             AF.Exp)
                        E.append(Ej)
                    # --- PE: denominator S (sum over 6 classes, dup halves) ---
                    lse = wk.tile([128, fd], BF16, tag="lse")
                    for s in range(nsub_i):
                        ssl = slice(base + s * sub, base + (s + 1) * sub)
                        esl = slice(s * sub, (s + 1) * sub)
                        for j in range(NPAIR):
                            nc.tensor.matmul(
                                s2all[:, ssl], wd_t[:], E[j][:, esl],
                                start=(j == 0), stop=(j == NPAIR - 1),
                            )
                    # --- ACT: lse = ln(S), one bank-spanning instr + CE accum ---
                    col = sb + 18 + chi
                    nc.scalar.activation(
                        lse[:, :ch_i], s2all[:, base:base + ch_i], AF.Ln,
                        accum_out=stats[:, col:col + 1],
                    )
                    # --- ACT: R2 = exp(-lse) = 1/S ---
                    R2 = wk.tile([128, fd], BF16, tag="R2")
                    nc.scalar.activation(R2[:, :ch_i], lse[:, :ch_i],
                                         AF.Exp, scale=-1.0)
                    # --- DVE: masks (4x), fused product+reduce for p_sum / TP ---
                    for j in range(NPAIR):
                        cc = chi * NPAIR + j
                        qj = wk.tile([128, fd], BF16, tag=f"q{j}")
                        nc.vector._custom_dve(
                            TTR_OP, out=qj[:, :ch_i], in0=E[j][:, :ch_i],
                            in1=R2[:, :ch_i], s0=0.0, s1=1.0,
                            accum_out=stats[:, sb + 0 + cc:sb + 1 + cc],
                        )
                        nc.vector._custom_dve(
                            EQSEL_OP, out=dummy[:, :ch_i],
                            in0=tg_t[:, b, sl_ch], in1=qj[:, :ch_i],
                            s0=cb_t[:, j, 0:1], s1=cb_t[:, j, 1:2],
                            accum_out=stats[:, sb + 9 + cc:sb + 10 + cc],
                        )
            nc.sync.dma_start(out_d.ap(), stats[:])
    nc.compile()
    return nc


def _weights():
    k = np.arange(128)
    wd = (k[:, None] % 64 == k[None, :] % 64).astype(NPBF16)
    cb = np.zeros((128, NPAIR, 2), dtype=np.float32)
    for j in range(NPAIR):
        cb[:64, j, 0] = 2 * j - 0.5
        cb[:64, j, 1] = 2 * j + 0.5
        cb[64:, j, 0] = 2 * j + 0.5
        cb[64:, j, 1] = 2 * j + 1.5
    return wd, cb


def _prep_core(logits_np, targets_np, cores, bpc, fd):
    """Build per-core input maps. logits (B,C,H,W) f32, targets (B,H,W) int."""
    wd, cb = _weights()
    lg = np.ascontiguousarray(logits_np.reshape(B, NPAIR, 128, fd)).astype(NPBF16)
    tghalf = targets_np.reshape(B, PHALF, fd).astype(NPBF16)
    tg = np.concatenate([tghalf, tghalf], axis=1)  # duplicate to both halves
    maps = []
    for c in range(cores):
        maps.append({
            "lg": np.ascontiguousarray(lg[c * bpc:(c + 1) * bpc]),
            "tg": np.ascontiguousarray(tg[c * bpc:(c + 1) * bpc]),
            "wd": wd, "cb": cb,
        })
    return maps


def _host_xt_sum(logits_np, targets_np):
    """Sum of target-class logits, bf16-rounded to match the device feed."""
    xt = np.take_along_axis(logits_np, targets_np[:, None].astype(np.int64),
                            axis=1)[:, 0]
    return float(xt.astype(NPBF16).astype(np.float64).sum())


def _host_t_sum(targets_np):
    """Per-image class histogram (exact integer counts)."""
    return np.stack([np.bincount(targets_np[i].ravel().astype(np.int64),
                                 minlength=C).astype(np.float64)
                     for i in range(B)])


def _finish(outs, bpc, xt_sum, t_sum):
    """Host combine: outs = list of [128, SCOL*bpc] f32 per core."""
    nchunks = {0: 2, 1: 1}
    p_sum = np.zeros((B, C)); tp = np.zeros((B, C))
    lse = np.zeros(B)
    for core, o in enumerate(outs):
        o = o.astype(np.float64)
        for b in range(bpc):
            img = core * bpc + b
            sb = SCOL * b
            nch = nchunks[b]
            for j in range(NPAIR):
                cols = [sb + chi * NPAIR + j for chi in range(nch)]
                for off, dst in ((0, p_sum), (9, tp)):
                    cc = [c + off for c in cols]
                    dst[img, 2 * j] = o[0:64, cc].sum()
                    dst[img, 2 * j + 1] = o[64:128, cc].sum()
            lse[img] = o[:, sb + 18:sb + 18 + nch].sum() / 2.0
    npx = B * HWPX
    ce = (lse.sum() - xt_sum) / npx
    dice = (2.0 * tp + 1e-8) / (p_sum + t_sum + 1e-8)
    dice_loss = np.mean(1.0 - dice)
    fp = p_sum - tp
    fn = t_sum - tp
    tversky = (tp + 1e-6) / (tp + FT_ALPHA * fn + FT_BETA * fp + 1e-6)
    ft_loss = np.mean((1.0 - tversky) ** FT_GAMMA)
    return np.float32(CE_W * ce + DICE_W * dice_loss + FT_W * ft_loss)


_CACHED = {}


def kernel(logits, targets):
    logits = np.asarray(logits, dtype=np.float32)
    targets = np.asarray(targets)
    if "nc" not in _CACHED:
        _CACHED["nc"] = _build()
    maps = _prep_core(logits, targets, NCORES, BPC, FD)
    res = run_bass_kernel_spmd(_CACHED["nc"], maps, list(range(NCORES)))
    outs = [res.results[i]["out"] for i in range(NCORES)]
    return _finish(outs, BPC, _host_xt_sum(logits, targets),
                   _host_t_sum(targets))


if __name__ == "__main__":
    rng = np.random.default_rng(0)
    logits = rng.standard_normal((B, C, H, W), dtype=np.float32)
    targets = rng.integers(0, C, size=(B, H, W)).astype(np.int64)
    got = kernel(logits, targets)

    # float64 numpy reference
    lg = logits.astype(np.float64)
    m = lg.max(axis=1, keepdims=True)
    e = np.exp(lg - m)
    s = e.sum(axis=1, keepdims=True)
    logp = lg - m - np.log(s)
    probs = e / s
    lp_t = np.take_along_axis(logp, targets[:, None], axis=1)[:, 0]
    ce = -lp_t.mean()
    oh = (targets[:, None] == np.arange(C)[None, :, None, None])
    tp = (probs * oh).sum(axis=(2, 3))
    p_sum = probs.sum(axis=(2, 3))
    t_sum = oh.sum(axis=(2, 3))
    dice = (2 * tp + 1e-8) / (p_sum + t_sum + 1e-8)
    dice_loss = np.mean(1 - dice)
    tv = (tp + 1e-6) / (tp + FT_ALPHA * (t_sum - tp) + FT_BETA * (p_sum - tp) + 1e-6)
    ft = np.mean((1 - tv) ** FT_GAMMA)
    want = CE_W * ce + DICE_W * dice_loss + FT_W * ft
    print("got", got, "want", want, "rel", abs(got - want) / abs(want))


# revision 13
# speedup vs baseline: 1.0929x; 1.0929x over previous
"""Combined CE + Dice + Focal-Tversky segmentation loss on 8 Trainium2 cores.

Layout: pure data parallel, 2 images per core. Per image, class planes are
packed in "class pair" tiles [128, 4096] bf16: pair j holds class 2j on
partitions 0-63 and class 2j+1 on partitions 64-127; partition p%64 holds
pixels [(p%64)*4096, (p%64+1)*4096).

Engine split (v2, tuned to measured TRN2 rates):
  ACT    Exp(logits), Ln(denominator)+CE-accum, Exp(-lse) -> 1/S
  PE     cross-class+cross-half softmax denominator sums only (PSUM accum)
  DVE    is_equal masks (tensor_scalar, 4x mode, fused t_sum accum),
         q = E*R2 and qm = q*M products (tensor_tensor, 2x mode),
         p_sum/TP column accumulators (tensor_scalar 4x accum passes)
  GPSIMD unused: its software-emulated ops run ~30us/instr and starve DVE
Host: CE's sum of target-class logits (gather on the bf16-rounded logits)
and the final scalar combine in float64.
"""

import os
import shutil
import sys
import tempfile

sys.path.insert(0, "/opt/trn_rl_repo")

import numpy as np

import concourse.bacc as bacc
import concourse.mybir as mybir
import concourse.tile as tile
from concourse.bass_utils import run_bass_kernel_spmd
from operator import add as _add

import concourse.dve_ops as _dom
from concourse.dve_ops import TENSOR_TENSOR_REDUCE as TTR_OP
from concourse.dve_ops import DveOp as _DveOp
from concourse.dve_spec import C0 as _C0, C1 as _C1, Spec as _Spec, \
    Src0 as _Src0, Src1 as _Src1, Zero as _Zero, select as _select


def _eq_ref(in0, in1, c0, c1, c2):
    body = np.where((in0.astype(np.float32) >= c0) & (in0 <= c1),
                    in1.astype(np.float32), 0.0)
    return body, body.sum(axis=-1, keepdims=True)


# Fused "select(lo<=tg<=hi, q, 0)" with free-dim sum accumulator: one DVE
# pass produces the masked-probability TP contribution without a separate
# mask tile.  Registered into the ant-dve custom-op table at import time.
EQSEL_OP = _DveOp(
    "EQ_SELECT_REDUCE_LOSS",
    _Spec(
        body=_select((_Src0 >= _C0) & (_Src0 <= _C1), _Src1, _Zero),
        accum=_add,
        accum_init=_Zero,
        reference=_eq_ref,
    ),
    subdim=False,
    uops_sha={"v3": "c6f19abea79fcfd4"},
)
if EQSEL_OP.name not in _dom._SUB_OPCODE_FOR_NAME:
    _dom.OPS.append(EQSEL_OP)
    _dom.CUSTOM_DVE_SPECS[EQSEL_OP.name] = EQSEL_OP.spec
    _dom._SUB_OPCODE_FOR_NAME[EQSEL_OP.name] = (
        _dom._CUSTOM_DVE_ROW_BASE + len(_dom.OPS) - 1)

B, C, H, W = 16, 6, 512, 512
NCORES = 8
BPC = B // NCORES  # images per core
HWPX = H * W  # 262144 pixels per image
PHALF = 64
FD = HWPX // PHALF  # 4096 free-dim columns per image
NPAIR = C // 2  # 3 class-pair tiles

CE_W, DICE_W, FT_W = 0.4, 0.4, 0.2
FT_ALPHA, FT_BETA, FT_GAMMA = 0.7, 0.3, 1.33

BF16 = mybir.dt.bfloat16
F32 = mybir.dt.float32
AF = mybir.ActivationFunctionType
ALU = mybir.AluOpType
NPBF16 = mybir.dt.np(BF16)

# tuning knobs
CH = 2048  # chunk free size
SUB = 512  # PSUM-bank sub-chunk for denominator matmuls
SCOL = 32  # stats columns reserved per image


def _pin_act_tables():
    """Point walrus at an act_info.json whose only exp/ln-bearing set is the
    combined natural_log_exp_and_others, so interleaved Exp/Ln ACTIVATEs do
    not thrash ACT_TABLE_LOADs."""
    if os.environ.get("BASS_ACT_ROOT_JSON_PATH"):
        return
    try:
        import json

        from neuronxcc.driver.Job import Job
        from neuronxcc.driver.jobs.support.FindActInfo import findActInfoFile

        src = findActInfoFile(Job.getPackageDir(), "gen3")
        if not src or not os.path.exists(src):
            return
        srcdir = os.path.dirname(src)
        dst = os.path.join(tempfile.gettempdir(), "act_root_lnexp")
        if not os.path.isdir(dst):
            tmp = dst + ".tmp"
            shutil.rmtree(tmp, ignore_errors=True)
            shutil.copytree(srcdir, tmp)
            info = json.load(open(os.path.join(tmp, "act_info.json")))
            keep = [s for s in info["act_func_sets"]
                    if s["name"] not in ("exp_and_others", "natural_log")]
            first = [s for s in keep if s["name"] == "natural_log_exp_and_others"]
            rest = [s for s in keep if s["name"] != "natural_log_exp_and_others"]
            info["act_func_sets"] = first + rest
            json.dump(info, open(os.path.join(tmp, "act_info.json"), "w"))
            os.replace(tmp, dst)
        os.environ["BASS_ACT_ROOT_JSON_PATH"] = os.path.join(dst, "act_info.json")
    except Exception:
        pass  # fall back to default tables; correctness unaffected


PIN_ACT_SET = os.environ.get("KPIN", "0") == "1"


def _build(fd=FD, ch=CH, sub=SUB, bpc=BPC):
    if PIN_ACT_SET:
        _pin_act_tables()
    nch = fd // ch
    nsub = ch // sub
    nc = bacc.Bacc("TRN2", target_bir_lowering=False, debug=False,
                   enable_asserts=False, num_devices=NCORES)

    lg_d = nc.dram_tensor("lg", [bpc, NPAIR, 128, fd], BF16, kind="ExternalInput")
    tg_d = nc.dram_tensor("tg", [bpc, 128, fd], BF16, kind="ExternalInput")
    wd_d = nc.dram_tensor("wd", [128, 128], BF16, kind="ExternalInput")
    cb_d = nc.dram_tensor("cb", [128, NPAIR, 2], F32, kind="ExternalInput")
    out_d = nc.dram_tensor("out", [128, SCOL * bpc], F32, kind="ExternalOutput")

    with tile.TileContext(nc) as tc:
        with (
            tc.tile_pool(name="inp", bufs=1) as inp,
            tc.tile_pool(name="wk", bufs=2) as wk,
            tc.tile_pool(name="acc", bufs=1) as accp,
            tc.tile_pool(name="ps", bufs=1, space="PSUM") as ps,
        ):
            wd_t = inp.tile([128, 128], BF16, tag="wd")
            nc.sync.dma_start(wd_t[:], wd_d.ap())
            cb_t = inp.tile([128, NPAIR, 2], F32, tag="cb")
            nc.sync.dma_start(cb_t[:], cb_d.ap())

            lg_t = inp.tile([128, bpc, NPAIR, fd], BF16, tag="lg")
            tg_t = inp.tile([128, bpc, fd], BF16, tag="tg")
            chunks = {b: ([(0, 512), (512, 1536), (2048, 2048)] if b == 0
                          else [(0, 2048), (2048, 2048)]) for b in range(bpc)}
            for b in range(bpc):
                for base, ch_i in chunks[b]:
                    for j in range(NPAIR):
                        nc.sync.dma_start(
                            lg_t[:, b, j, base:base + ch_i],
                            lg_d.ap()[b, j, :, base:base + ch_i])
                    if base == 0:
                        nc.sync.dma_start(tg_t[:, b, :], tg_d.ap()[b])

            stats = accp.tile([128, SCOL * bpc], F32, tag="stats")
            nc.vector.memset(stats[:], 0.0)
            dummy = accp.tile([128, fd], BF16, tag="dummy")
            s2all = ps.tile([128, fd], F32, tag="s2")

            for b in range(bpc):
                sb = SCOL * b
                subs_done = 0
                for chi, (base, ch_i) in enumerate(chunks[b]):
                    sl_ch = slice(base, base + ch_i)
                    nsub_i = ch_i // sub
                    # --- ACT: exponentials of all class pairs ---
                    E = []
                    for j in range(NPAIR):
                        Ej = wk.tile([128, fd], BF16, tag=f"E{j}")
                        nc.scalar.activation(Ej[:, :ch_i], lg_t[:, b, j, sl_ch],
                                             AF.Exp)
                        E.append(Ej)
                    # --- PE: denominator S (sum over 6 classes, dup halves) ---
                    lse = wk.tile([128, fd], BF16, tag="lse")
                    for s in range(nsub_i):
                        ssl = slice(base + s * sub, base + (s + 1) * sub)
                        esl = slice(s * sub, (s + 1) * sub)
                        for j in range(NPAIR):
                            nc.tensor.matmul(
                                s2all[:, ssl], wd_t[:], E[j][:, esl],
                                start=(j == 0), stop=(j == NPAIR - 1),
                            )
                    # --- ACT: lse = ln(S), one bank-spanning instr + CE accum ---
                    col = sb + 18 + chi
                    nc.scalar.activation(
                        lse[:, :ch_i], s2all[:, base:base + ch_i], AF.Ln,
                        accum_out=stats[:, col:col + 1],
                    )
                    # --- ACT: R2 = exp(-lse) = 1/S ---
                    R2 = wk.tile([128, fd], BF16, tag="R2")
                    nc.scalar.activation(R2[:, :ch_i], lse[:, :ch_i],
                                         AF.Exp, scale=-1.0)
                    # --- DVE: masks (4x), fused product+reduce for p_sum / TP ---
                    for j in range(NPAIR):
                        cc = chi * NPAIR + j
                        qj = wk.tile([128, fd], BF16, tag=f"q{j}")
                        nc.vector._custom_dve(
                            TTR_OP, out=qj[:, :ch_i], in0=E[j][:, :ch_i],
                            in1=R2[:, :ch_i], s0=0.0, s1=1.0,
                            accum_out=stats[:, sb + 0 + cc:sb + 1 + cc],
                        )
                        nc.vector._custom_dve(
                            EQSEL_OP, out=dummy[:, :ch_i],
                            in0=tg_t[:, b, sl_ch], in1=qj[:, :ch_i],
                            s0=cb_t[:, j, 0:1], s1=cb_t[:, j, 1:2],
                            accum_out=stats[:, sb + 9 + cc:sb + 10 + cc],
                        )
            nc.sync.dma_start(out_d.ap(), stats[:])
    nc.compile()
    return nc


def _weights():
    k = np.arange(128)
    wd = (k[:, None] % 64 == k[None, :] % 64).astype(NPBF16)
    cb = np.zeros((128, NPAIR, 2), dtype=np.float32)
    for j in range(NPAIR):
        cb[:64, j, 0] = 2 * j - 0.5
        cb[:64, j, 1] = 2 * j + 0.5
        cb[64:, j, 0] = 2 * j + 0.5
        cb[64:, j, 1] = 2 * j + 1.5
    return wd, cb


def _prep_core(logits_np, targets_np, cores, bpc, fd):
    """Build per-core input maps. logits (B,C,H,W) f32, targets (B,H,W) int."""
    wd, cb = _weights()
    lg = np.ascontiguousarray(logits_np.reshape(B, NPAIR, 128, fd)).astype(NPBF16)
    tghalf = targets_np.reshape(B, PHALF, fd).astype(NPBF16)
    tg = np.concatenate([tghalf, tghalf], axis=1)  # duplicate to both halves
    maps = []
    for c in range(cores):
        maps.append({
            "lg": np.ascontiguousarray(lg[c * bpc:(c + 1) * bpc]),
            "tg": np.ascontiguousarray(tg[c * bpc:(c + 1) * bpc]),
            "wd": wd, "cb": cb,
        })
    return maps


def _host_xt_sum(logits_np, targets_np):
    """Sum of target-class logits, bf16-rounded to match the device feed."""
    xt = np.take_along_axis(logits_np, targets_np[:, None].astype(np.int64),
                            axis=1)[:, 0]
    return float(xt.astype(NPBF16).astype(np.float64).sum())


def _host_t_sum(targets_np):
    """Per-image class histogram (exact integer counts)."""
    return np.stack([np.bincount(targets_np[i].ravel().astype(np.int64),
                                 minlength=C).astype(np.float64)
                     for i in range(B)])


def _finish(outs, bpc, xt_sum, t_sum):
    """Host combine: outs = list of [128, SCOL*bpc] f32 per core."""
    nchunks = {0: 3, 1: 2}
    p_sum = np.zeros((B, C)); tp = np.zeros((B, C))
    lse = np.zeros(B)
    for core, o in enumerate(outs):
        o = o.astype(np.float64)
        for b in range(bpc):
            img = core * bpc + b
            sb = SCOL * b
            nch = nchunks[b]
            for j in range(NPAIR):
                cols = [sb + chi * NPAIR + j for chi in range(nch)]
                for off, dst in ((0, p_sum), (9, tp)):
                    cc = [c + off for c in cols]
                    dst[img, 2 * j] = o[0:64, cc].sum()
                    dst[img, 2 * j + 1] = o[64:128, cc].sum()
            lse[img] = o[:, sb + 18:sb + 18 + nch].sum() / 2.0
    npx = B * HWPX
    ce = (lse.sum() - xt_sum) / npx
    dice = (2.0 * tp + 1e-8) / (p_sum + t_sum + 1e-8)
    dice_loss = np.mean(1.0 - dice)
    fp = p_sum - tp
    fn = t_sum - tp
    tversky = (tp + 1e-6) / (tp + FT_ALPHA * fn + FT_BETA * fp + 1e-6)
    ft_loss = np.mean((1.0 - tversky) ** FT_GAMMA)
    return np.float32(CE_W * ce + DICE_W * dice_loss + FT_W * ft_loss)


_CACHED = {}


def kernel(logits, targets):
    logits = np.asarray(logits, dtype=np.float32)
    targets = np.asarray(targets)
    if "nc" not in _CACHED:
        _CACHED["nc"] = _build()
    maps = _prep_core(logits, targets, NCORES, BPC, FD)
    res = run_bass_kernel_spmd(_CACHED["nc"], maps, list(range(NCORES)))
    outs = [res.results[i]["out"] for i in range(NCORES)]
    return _finish(outs, BPC, _host_xt_sum(logits, targets),
                   _host_t_sum(targets))


if __name__ == "__main__":
    rng = np.random.default_rng(0)
    logits = rng.standard_normal((B, C, H, W), dtype=np.float32)
    targets = rng.integers(0, C, size=(B, H, W)).astype(np.int64)
    got = kernel(logits, targets)

    # float64 numpy reference
    lg = logits.astype(np.float64)
    m = lg.max(axis=1, keepdims=True)
    e = np.exp(lg - m)
    s = e.sum(axis=1, keepdims=True)
    logp = lg - m - np.log(s)
    probs = e / s
    lp_t = np.take_along_axis(logp, targets[:, None], axis=1)[:, 0]
    ce = -lp_t.mean()
    oh = (targets[:, None] == np.arange(C)[None, :, None, None])
    tp = (probs * oh).sum(axis=(2, 3))
    p_sum = probs.sum(axis=(2, 3))
    t_sum = oh.sum(axis=(2, 3))
    dice = (2 * tp + 1e-8) / (p_sum + t_sum + 1e-8)
    dice_loss = np.mean(1 - dice)
    tv = (tp + 1e-6) / (tp + FT_ALPHA * (t_sum - tp) + FT_BETA * (p_sum - tp) + 1e-6)
    ft = np.mean((1 - tv) ** FT_GAMMA)
    want = CE_W * ce + DICE_W * dice_loss + FT_W * ft
    print("got", got, "want", want, "rel", abs(got - want) / abs(want))
